# revision 19
# baseline (speedup 1.0000x reference)
"""Trainium2 Bass kernel for the scatter_memory GRU memory-update module.

Computation (torch GRUCell semantics, chunk order r, z, n):
    current = memory[node_ids]                       # [B, H] gather
    gi = messages @ W_ih.T + b_ih ; gh = current @ W_hh.T + b_hh
    r = sigmoid(gi_r + gh_r) ; z = sigmoid(gi_z + gh_z)
    n = tanh(gi_n + r * gh_n)
    updated = (1 - z) * n + z * current
    new_memory = memory.at[node_ids].set(updated)    # scatter

Distribution: the B updated rows are sharded contiguously across 8
NeuronCores.  The gather/scatter over the 500k-row table and the
feature-major transposes run on the host; each core runs the GRU math on
its own [H, B/8] shard (feature dim H=128 sits on the SBUF partition
axis, so the GRU biases become per-partition vectors that fuse into the
ScalarEngine activation ops for free).

Device schedule (per 1024-column quantum):
  PE   : p_r = W_ih,r*x + W_hh,r*h ; p_z likewise ; C = W_hh,n*h
         (and, one quantum later, C += W_ih,n*x with start=False)
  ACT  : r = sigmoid(p_r + b_r), z = sigmoid(p_z + b_z)  [1024-wide]
         n = tanh(C + b_ihn)  after the delayed i_n accumulation
  DVE  : t = (C + b_hhn) * r written in place into C (the only 1x op),
         then the blend as three all-bf16 SBUF STTs at 4x rate:
         zh = z*h ; v = (z-1)*n ; o = zh - v
PSUM: p_r x1, p_z x1, C x2 = exactly 8 banks.
"""

import os
import sys

import numpy as np

for _p in ("/opt/trn_rl_repo", "/root/.axon_site/_ro/trn_rl_repo"):
    if os.path.isdir(_p) and _p not in sys.path:
        sys.path.insert(0, _p)

import ml_dtypes
from contextlib import ExitStack

import concourse.bass as bass
import concourse.tile as tile
from concourse import mybir
from concourse.bass_utils import run_bass_kernel_spmd

BF16 = ml_dtypes.bfloat16
import json as _json

N_CORES = 8
H = 128
Q = 1024             # batch columns per quantum (2 PSUM banks of fp32)

# exposed for test harnesses
LAST_RESULT = None

_NC_CACHE = {}


def _split_sync_waits(bir: dict) -> dict:
    """Hoist extra per-instruction semaphore waits into standalone
    EventSemaphore instructions.

    The walrus build in this container encodes at most ONE sync wait per
    instruction ("Too many sync wait commands" otherwise); Tile attaches
    one wait per dependency.  An engine-level standalone wait immediately
    before the instruction is semantically identical (the engine stalls
    either way), so keep the last wait inline and hoist the rest.
    """
    n = 0
    for fn in bir.get("functions", []):
        for blk in fn.get("blocks", []):
            out = []
            for inst in blk.get("instructions", []):
                si = inst.get("sync_info") or {}
                ow = si.get("on_wait") or []
                if len(ow) > 1:
                    for w in ow[:-1]:
                        n += 1
                        out.append({
                            "debug": inst.get("debug", 0),
                            "engine": inst["engine"],
                            "ins": [],
                            "outs": [],
                            "name": f"hoistw_{n}_{inst['name']}",
                            "opcode": "EventSemaphore",
                            "sync_info": {"on_update": [], "on_wait": [w]},
                        })
                    si["on_wait"] = [ow[-1]]
                out.append(inst)
            blk["instructions"] = out
    return bir


def _patch_json(nc: bass.Bass) -> None:
    orig = nc.to_json_bytes

    def patched() -> bytes:
        return _json.dumps(_split_sync_waits(_json.loads(orig()))).encode()

    nc.to_json_bytes = patched


def _quanta(bpc: int) -> list[tuple[int, int]]:
    """(offset, size) quanta: tapered opening (fast pipeline fill while the
    PE p-state ramps), full 1024s in the middle, tapered tail (short final
    serial chain)."""
    out = []
    pos = 0
    for s in (256, 256, 512, 512):
        if bpc - pos < Q:
            break
        out.append((pos, s))
        pos += s
    while bpc - pos >= Q + 512:
        out.append((pos, Q))
        pos += Q
    for s in (512, 512):
        if pos >= bpc:
            break
        csz = min(s, bpc - pos)
        out.append((pos, csz))
        pos += csz
    while pos < bpc:
        csz = min(512, bpc - pos)
        out.append((pos, csz))
        pos += csz
    return out


def _build_nc(bpc: int) -> bass.Bass:
    """Bass program for one core: GRU over a [H, bpc] feature-major shard."""
    f32 = mybir.dt.float32
    bf16 = mybir.dt.bfloat16
    sig = mybir.ActivationFunctionType.Sigmoid
    tanh = mybir.ActivationFunctionType.Tanh
    add_op = mybir.AluOpType.add
    sub_op = mybir.AluOpType.subtract
    mult_op = mybir.AluOpType.mult

    nc = bass.Bass()
    xT = nc.declare_dram_parameter("xT", [H, bpc], bf16, isOutput=False)
    hT = nc.declare_dram_parameter("hT", [H, bpc], bf16, isOutput=False)
    w_ihT = nc.declare_dram_parameter("w_ihT", [H, 3 * H], bf16, isOutput=False)
    w_hhT = nc.declare_dram_parameter("w_hhT", [H, 3 * H], bf16, isOutput=False)
    # bias columns: 0 = b_ih_r + b_hh_r, 1 = b_ih_z + b_hh_z, 2 = b_hh_n, 3 = b_ih_n
    biases = nc.declare_dram_parameter("biases", [H, 4], f32, isOutput=False)
    outT = nc.declare_dram_parameter("outT", [H, bpc], bf16, isOutput=True)

    qs = _quanta(bpc)
    nq = len(qs)

    # input DMA chunks: first quantum alone (fast pipeline start), then
    # pairs of quanta; a quantum never straddles a chunk boundary.
    chunks = []          # (offset, size)
    qchunk = []          # quantum index -> chunk index
    ci = 0
    i = 0
    while i < nq:
        take = 1 if i < 4 else min(2, nq - i)
        off = qs[i][0]
        size = sum(s for _, s in qs[i : i + take])
        chunks.append((off, size))
        for _ in range(take):
            qchunk.append(ci)
        ci += 1
        i += take

    # output DMA chunks: one per quantum so stores stream instead of
    # bunching at the drain
    ochunks = list(qs)
    oq = [(i, 0) for i in range(nq)]

    with ExitStack() as ctx:
        tc = ctx.enter_context(tile.TileContext(nc))
        singles = ctx.enter_context(tc.tile_pool(name="singles", bufs=1))
        io = ctx.enter_context(tc.tile_pool(name="io", bufs=len(chunks)))
        mids = ctx.enter_context(tc.tile_pool(name="mids", bufs=3))
        wide = ctx.enter_context(tc.tile_pool(name="wide", bufs=3))
        psum = ctx.enter_context(tc.tile_pool(name="psum", bufs=1, space="PSUM"))

        # one-time loads via the sync-engine HWDGE queues (fast, lands first)
        b_sb = singles.tile([H, 4], f32)
        nc.sync.dma_start(out=b_sb, in_=biases[:, :])
        w_ih_sb = singles.tile([H, 3 * H], bf16)
        nc.sync.dma_start(out=w_ih_sb, in_=w_ihT[:, :])
        w_hh_sb = singles.tile([H, 3 * H], bf16)
        nc.sync.dma_start(out=w_hh_sb, in_=w_hhT[:, :])

        # dummy activations fire the ~1.3us ACT table load(s) immediately so
        # they overlap the DMA ramp instead of stalling the first real sigmoid
        warm_sb = singles.tile([H, 2], f32)
        nc.scalar.activation(out=warm_sb[:, 0:1], in_=b_sb[:, 0:1],
                             func=sig, bias=0.0, scale=1.0)
        nc.scalar.activation(out=warm_sb[:, 1:2], in_=b_sb[:, 0:1],
                             func=tanh, bias=0.0, scale=1.0)

        # issue every input chunk DMA up front; the 16 HWDGE queues stream
        # while compute follows behind
        x_tiles, h_tiles = [], []
        for c0, csz in chunks:
            x_sb = io.tile([H, 2 * Q], bf16, tag="x", name="x_sb")[:, :csz]
            h_sb = io.tile([H, 2 * Q], bf16, tag="h", name="h_sb")[:, :csz]
            nc.sync.dma_start(out=x_sb, in_=xT[:, c0 : c0 + csz])
            nc.sync.dma_start(out=h_sb, in_=hT[:, c0 : c0 + csz])
            x_tiles.append(x_sb)
            h_tiles.append(h_sb)

        W_R, W_Z, W_N = slice(0, H), slice(H, 2 * H), slice(2 * H, 3 * H)

        state = {}           # per-quantum tiles needed by the delayed stages
        o_tiles = [None] * len(ochunks)

        def emit_front(qi):
            """PE r/z/hn matmuls + ACT sigmoids + DVE t for quantum qi."""
            q0, qsz = qs[qi]
            cki = qchunk[qi]
            x_sb, h_sb = x_tiles[cki], h_tiles[cki]
            xo = q0 - chunks[cki][0]
            halves = [(xo + p, min(512, qsz - p)) for p in range(0, qsz, 512)]

            # always allocate full-width tiles (slice for the tail quantum)
            # so every PSUM tag has one uniform shape across the program
            r_ps = psum.tile([H, Q], f32, tag="r", bufs=1, name="r_ps")[:, :qsz]
            z_ps = psum.tile([H, Q], f32, tag="z", bufs=1, name="z_ps")[:, :qsz]
            c_ps = psum.tile([H, Q], f32, tag="c", bufs=2, name="c_ps")[:, :qsz]

            # paired by stationary weight so the PE array reloads less often
            for h0, hsz in halves:
                nc.tensor.matmul(r_ps[:, h0 - xo : h0 - xo + hsz],
                                 w_ih_sb[:, W_R], x_sb[:, h0 : h0 + hsz],
                                 start=True, stop=False)
            for h0, hsz in halves:
                nc.tensor.matmul(r_ps[:, h0 - xo : h0 - xo + hsz],
                                 w_hh_sb[:, W_R], h_sb[:, h0 : h0 + hsz],
                                 start=False, stop=True)
            for h0, hsz in halves:
                nc.tensor.matmul(z_ps[:, h0 - xo : h0 - xo + hsz],
                                 w_ih_sb[:, W_Z], x_sb[:, h0 : h0 + hsz],
                                 start=True, stop=False)
            for h0, hsz in halves:
                nc.tensor.matmul(z_ps[:, h0 - xo : h0 - xo + hsz],
                                 w_hh_sb[:, W_Z], h_sb[:, h0 : h0 + hsz],
                                 start=False, stop=True)
            for h0, hsz in halves:
                nc.tensor.matmul(c_ps[:, h0 - xo : h0 - xo + hsz],
                                 w_hh_sb[:, W_N], h_sb[:, h0 : h0 + hsz],
                                 start=True, stop=True)

            r_sb = mids.tile([H, Q], bf16, tag="r_sb", bufs=2, name="r_sb")[:, :qsz]
            z_sb = mids.tile([H, Q], bf16, tag="z_sb", bufs=2, name="z_sb")[:, :qsz]
            nc.scalar.activation(out=r_sb, in_=r_ps, func=sig,
                                 bias=b_sb[:, 0:1], scale=1.0)
            nc.scalar.activation(out=z_sb, in_=z_ps, func=sig,
                                 bias=b_sb[:, 1:2], scale=1.0)

            # t = (h_n + b_hhn) * r written IN PLACE into the C bank; the
            # delayed i_n matmul then accumulates on top of it.
            nc.vector.scalar_tensor_tensor(
                out=c_ps, in0=c_ps, scalar=b_sb[:, 2:3], in1=r_sb,
                op0=add_op, op1=mult_op)

            state[qi] = (c_ps, z_sb, halves, x_sb, h_sb, xo)

        def emit_back(qi):
            """Delayed i_n accumulation + tanh + blend + out DMA for qi."""
            q0, qsz = qs[qi]
            c_ps, z_sb, halves, x_sb, h_sb, xo = state.pop(qi)

            for h0, hsz in halves:
                nc.tensor.matmul(c_ps[:, h0 - xo : h0 - xo + hsz],
                                 w_ih_sb[:, W_N], x_sb[:, h0 : h0 + hsz],
                                 start=False, stop=True, skip_group_check=True)

            n_sb = mids.tile([H, Q], bf16, tag="n_sb", bufs=2, name="n_sb")[:, :qsz]
            nc.scalar.activation(out=n_sb, in_=c_ps, func=tanh,
                                 bias=b_sb[:, 3:4], scale=1.0)

            oc, oin = oq[qi]
            o_sb = wide.tile([H, Q], bf16, tag="o", bufs=3,
                             name="o_sb")[:, :qsz]

            # o = n + z*(h-n) as three all-bf16 tensor_tensor ops (DVE 2x
            # rate); the subtract alternates onto GpSimd to offload DVE
            d_sb = mids.tile([H, Q], bf16, tag="d", bufs=2, name="d_sb")[:, :qsz]
            m_sb = mids.tile([H, Q], bf16, tag="m", bufs=2, name="m_sb")[:, :qsz]
            d_eng = nc.gpsimd if (qi % 2 == 0) else nc.vector
            d_eng.tensor_sub(out=d_sb, in0=h_sb[:, xo : xo + qsz], in1=n_sb)
            nc.vector.tensor_mul(out=m_sb, in0=z_sb, in1=d_sb)
            nc.vector.tensor_add(out=o_sb[:, oin : oin + qsz], in0=m_sb,
                                 in1=n_sb)

            nc.sync.dma_start(out=outT[:, q0 : q0 + qsz], in_=o_sb)

        for qi in range(nq):
            if qi > 0:
                emit_back(qi - 1)
            emit_front(qi)
        emit_back(nq - 1)

    _patch_json(nc)
    return nc


def _get_nc(bpc: int) -> bass.Bass:
    if bpc not in _NC_CACHE:
        _NC_CACHE[bpc] = _build_nc(bpc)
    return _NC_CACHE[bpc]


def kernel(node_ids, messages, memory, W_ih, W_hh, b_ih, b_hh):
    global LAST_RESULT
    node_ids = np.asarray(node_ids)
    messages = np.asarray(messages, dtype=np.float32)
    memory = np.asarray(memory, dtype=np.float32)
    W_ih = np.asarray(W_ih, dtype=np.float32)
    W_hh = np.asarray(W_hh, dtype=np.float32)
    b_ih = np.asarray(b_ih, dtype=np.float32)
    b_hh = np.asarray(b_hh, dtype=np.float32)

    B = node_ids.shape[0]
    per = -(-B // N_CORES)                       # rows per core
    bpc = max(per, 512)
    nc = _get_nc(bpc)

    current = memory[node_ids]                   # [B, H] host gather

    w_ihT = np.ascontiguousarray(W_ih.T).astype(BF16)
    w_hhT = np.ascontiguousarray(W_hh.T).astype(BF16)
    bias = np.empty((H, 4), dtype=np.float32)
    bias[:, 0] = b_ih[0:H] + b_hh[0:H]
    bias[:, 1] = b_ih[H : 2 * H] + b_hh[H : 2 * H]
    bias[:, 2] = b_hh[2 * H : 3 * H]
    bias[:, 3] = b_ih[2 * H : 3 * H]

    in_maps = []
    for c in range(N_CORES):
        lo = c * per
        hi = min(lo + per, B)
        if hi - lo == bpc:
            xT = np.ascontiguousarray(messages[lo:hi].T).astype(BF16)
            hT = np.ascontiguousarray(current[lo:hi].T).astype(BF16)
        else:
            xT = np.zeros((H, bpc), dtype=BF16)
            hT = np.zeros((H, bpc), dtype=BF16)
            if hi > lo:
                xT[:, : hi - lo] = messages[lo:hi].T
                hT[:, : hi - lo] = current[lo:hi].T
        in_maps.append({
            "xT": xT, "hT": hT,
            "w_ihT": w_ihT, "w_hhT": w_hhT, "biases": bias,
        })

    res = run_bass_kernel_spmd(nc, in_maps, list(range(N_CORES)))
    LAST_RESULT = res

    updated = np.empty((B, H), dtype=np.float32)
    for c in range(N_CORES):
        lo = c * per
        hi = min(lo + per, B)
        if hi > lo:
            updated[lo:hi] = res.results[c]["outT"][:, : hi - lo].T.astype(np.float32)

    new_memory = memory.copy()
    new_memory[node_ids] = updated
    return new_memory


# revision 24
# speedup vs baseline: 1.0550x; 1.0550x over previous
"""Trainium2 Bass kernel for the scatter_memory GRU memory-update module.

Computation (torch GRUCell semantics, chunk order r, z, n):
    current = memory[node_ids]                       # [B, H] gather
    gi = messages @ W_ih.T + b_ih ; gh = current @ W_hh.T + b_hh
    r = sigmoid(gi_r + gh_r) ; z = sigmoid(gi_z + gh_z)
    n = tanh(gi_n + r * gh_n)
    updated = (1 - z) * n + z * current
    new_memory = memory.at[node_ids].set(updated)    # scatter

Distribution: the B updated rows are sharded contiguously across 8
NeuronCores.  The gather/scatter over the 500k-row table and the
feature-major transposes run on the host; each core runs the GRU math on
its own [H, B/8] shard (feature dim H=128 sits on the SBUF partition
axis, so the GRU biases become per-partition vectors that fuse into the
ScalarEngine activation ops for free).

Device schedule (per 1024-column quantum):
  PE   : p_r = W_ih,r*x + W_hh,r*h ; p_z likewise ; C = W_hh,n*h
         (and, one quantum later, C += W_ih,n*x with start=False)
  ACT  : r = sigmoid(p_r + b_r), z = sigmoid(p_z + b_z)  [1024-wide]
         n = tanh(C + b_ihn)  after the delayed i_n accumulation
  DVE  : t = (C + b_hhn) * r written in place into C (the only 1x op),
         then the blend as three all-bf16 SBUF STTs at 4x rate:
         zh = z*h ; v = (z-1)*n ; o = zh - v
PSUM: p_r x1, p_z x1, C x2 = exactly 8 banks.
"""

import os
import sys

import numpy as np

for _p in ("/opt/trn_rl_repo", "/root/.axon_site/_ro/trn_rl_repo"):
    if os.path.isdir(_p) and _p not in sys.path:
        sys.path.insert(0, _p)

import ml_dtypes
from contextlib import ExitStack

import concourse.bass as bass
import concourse.tile as tile
from concourse import mybir
from concourse.bass_utils import run_bass_kernel_spmd

BF16 = ml_dtypes.bfloat16
import json as _json

N_CORES = 8
H = 128
Q = 1024             # batch columns per quantum (2 PSUM banks of fp32)

# exposed for test harnesses
LAST_RESULT = None

_NC_CACHE = {}


def _split_sync_waits(bir: dict) -> dict:
    """Hoist extra per-instruction semaphore waits into standalone
    EventSemaphore instructions.

    The walrus build in this container encodes at most ONE sync wait per
    instruction ("Too many sync wait commands" otherwise); Tile attaches
    one wait per dependency.  An engine-level standalone wait immediately
    before the instruction is semantically identical (the engine stalls
    either way), so keep the last wait inline and hoist the rest.
    """
    n = 0
    for fn in bir.get("functions", []):
        for blk in fn.get("blocks", []):
            out = []
            for inst in blk.get("instructions", []):
                si = inst.get("sync_info") or {}
                ow = si.get("on_wait") or []
                if len(ow) > 1:
                    for w in ow[:-1]:
                        n += 1
                        out.append({
                            "debug": inst.get("debug", 0),
                            "engine": inst["engine"],
                            "ins": [],
                            "outs": [],
                            "name": f"hoistw_{n}_{inst['name']}",
                            "opcode": "EventSemaphore",
                            "sync_info": {"on_update": [], "on_wait": [w]},
                        })
                    si["on_wait"] = [ow[-1]]
                out.append(inst)
            blk["instructions"] = out
    return bir


def _patch_json(nc: bass.Bass) -> None:
    orig = nc.to_json_bytes

    def patched() -> bytes:
        return _json.dumps(_split_sync_waits(_json.loads(orig()))).encode()

    nc.to_json_bytes = patched


def _quanta(bpc: int) -> list[tuple[int, int]]:
    """(offset, size) quanta: tapered opening (fast pipeline fill while the
    PE p-state ramps), full 1024s in the middle, tapered tail (short final
    serial chain)."""
    out = []
    pos = 0
    if bpc > Q:
        out.append((0, 512))
        pos = 512
    while bpc - pos >= Q:
        out.append((pos, Q))
        pos += Q
    if pos < bpc:
        out.append((pos, bpc - pos))
    return out


def _build_nc(bpc: int) -> bass.Bass:
    """Bass program for one core: GRU over a [H, bpc] feature-major shard."""
    f32 = mybir.dt.float32
    bf16 = mybir.dt.bfloat16
    sig = mybir.ActivationFunctionType.Sigmoid
    tanh = mybir.ActivationFunctionType.Tanh
    add_op = mybir.AluOpType.add
    sub_op = mybir.AluOpType.subtract
    mult_op = mybir.AluOpType.mult

    nc = bass.Bass()
    xT = nc.declare_dram_parameter("xT", [H, bpc], bf16, isOutput=False)
    hT = nc.declare_dram_parameter("hT", [H, bpc], bf16, isOutput=False)
    w_ihT = nc.declare_dram_parameter("w_ihT", [H, 3 * H], bf16, isOutput=False)
    w_hhT = nc.declare_dram_parameter("w_hhT", [H, 3 * H], bf16, isOutput=False)
    # bias columns: 0 = b_ih_r + b_hh_r, 1 = b_ih_z + b_hh_z, 2 = b_hh_n, 3 = b_ih_n
    biases = nc.declare_dram_parameter("biases", [H, 4], f32, isOutput=False)
    outT = nc.declare_dram_parameter("outT", [H, bpc], bf16, isOutput=True)

    qs = _quanta(bpc)
    nq = len(qs)

    # input DMA chunks: first quantum alone (fast pipeline start), then
    # pairs of quanta; a quantum never straddles a chunk boundary.
    chunks = []          # (offset, size)
    qchunk = []          # quantum index -> chunk index
    ci = 0
    i = 0
    while i < nq:
        take = 1 if i == 0 else min(2, nq - i)
        off = qs[i][0]
        size = sum(s for _, s in qs[i : i + take])
        chunks.append((off, size))
        for _ in range(take):
            qchunk.append(ci)
        ci += 1
        i += take

    # output DMA chunks: one per quantum so stores stream instead of
    # bunching at the drain
    ochunks = list(qs)
    oq = [(i, 0) for i in range(nq)]

    with ExitStack() as ctx:
        tc = ctx.enter_context(tile.TileContext(nc))
        singles = ctx.enter_context(tc.tile_pool(name="singles", bufs=1))
        io = ctx.enter_context(tc.tile_pool(name="io", bufs=len(chunks)))
        mids = ctx.enter_context(tc.tile_pool(name="mids", bufs=3))
        wide = ctx.enter_context(tc.tile_pool(name="wide", bufs=3))
        psum = ctx.enter_context(tc.tile_pool(name="psum", bufs=1, space="PSUM"))

        # one-time loads via the sync-engine HWDGE queues (fast, lands first)
        b_sb = singles.tile([H, 4], f32)
        nc.sync.dma_start(out=b_sb, in_=biases[:, :])
        w_ih_sb = singles.tile([H, 3 * H], bf16)
        nc.sync.dma_start(out=w_ih_sb, in_=w_ihT[:, :])
        w_hh_sb = singles.tile([H, 3 * H], bf16)
        nc.sync.dma_start(out=w_hh_sb, in_=w_hhT[:, :])

        # dummy activations fire the ~1.3us ACT table load(s) immediately so
        # they overlap the DMA ramp instead of stalling the first real sigmoid
        warm_sb = singles.tile([H, 2], f32)
        nc.scalar.activation(out=warm_sb[:, 0:1], in_=b_sb[:, 0:1],
                             func=sig, bias=0.0, scale=1.0)
        nc.scalar.activation(out=warm_sb[:, 1:2], in_=b_sb[:, 0:1],
                             func=tanh, bias=0.0, scale=1.0)

        # dummy matmul chain keeps the PE continuously busy through the DMA
        # ramp so its p-state reaches full clock before real work arrives
        pewarm = singles.tile([H, 512], bf16)
        nc.vector.memset(pewarm, 0.0)
        warm_ps = psum.tile([H, Q], f32, tag="r", bufs=1, name="warm_ps")
        for _ in range(8):
            nc.tensor.matmul(warm_ps[:, 0:512], pewarm[:, 0:H], pewarm,
                             start=True, stop=True)

        # issue every input chunk DMA up front; the 16 HWDGE queues stream
        # while compute follows behind
        x_tiles, h_tiles = [], []
        for c0, csz in chunks:
            x_sb = io.tile([H, 2 * Q], bf16, tag="x", name="x_sb")[:, :csz]
            h_sb = io.tile([H, 2 * Q], bf16, tag="h", name="h_sb")[:, :csz]
            nc.sync.dma_start(out=x_sb, in_=xT[:, c0 : c0 + csz])
            nc.sync.dma_start(out=h_sb, in_=hT[:, c0 : c0 + csz])
            x_tiles.append(x_sb)
            h_tiles.append(h_sb)

        W_R, W_Z, W_N = slice(0, H), slice(H, 2 * H), slice(2 * H, 3 * H)

        state = {}           # per-quantum tiles needed by the delayed stages
        o_tiles = [None] * len(ochunks)

        def emit_front(qi):
            """PE r/z/hn matmuls + ACT sigmoids + DVE t for quantum qi."""
            q0, qsz = qs[qi]
            cki = qchunk[qi]
            x_sb, h_sb = x_tiles[cki], h_tiles[cki]
            xo = q0 - chunks[cki][0]
            halves = [(xo + p, min(512, qsz - p)) for p in range(0, qsz, 512)]

            # always allocate full-width tiles (slice for the tail quantum)
            # so every PSUM tag has one uniform shape across the program
            r_ps = psum.tile([H, Q], f32, tag="r", bufs=1, name="r_ps")[:, :qsz]
            z_ps = psum.tile([H, Q], f32, tag="z", bufs=1, name="z_ps")[:, :qsz]
            c_ps = psum.tile([H, Q], f32, tag="c", bufs=2, name="c_ps")[:, :qsz]

            # paired by stationary weight so the PE array reloads less often
            for h0, hsz in halves:
                nc.tensor.matmul(r_ps[:, h0 - xo : h0 - xo + hsz],
                                 w_ih_sb[:, W_R], x_sb[:, h0 : h0 + hsz],
                                 start=True, stop=False)
            for h0, hsz in halves:
                nc.tensor.matmul(r_ps[:, h0 - xo : h0 - xo + hsz],
                                 w_hh_sb[:, W_R], h_sb[:, h0 : h0 + hsz],
                                 start=False, stop=True)
            for h0, hsz in halves:
                nc.tensor.matmul(z_ps[:, h0 - xo : h0 - xo + hsz],
                                 w_ih_sb[:, W_Z], x_sb[:, h0 : h0 + hsz],
                                 start=True, stop=False)
            for h0, hsz in halves:
                nc.tensor.matmul(z_ps[:, h0 - xo : h0 - xo + hsz],
                                 w_hh_sb[:, W_Z], h_sb[:, h0 : h0 + hsz],
                                 start=False, stop=True)
            for h0, hsz in halves:
                nc.tensor.matmul(c_ps[:, h0 - xo : h0 - xo + hsz],
                                 w_hh_sb[:, W_N], h_sb[:, h0 : h0 + hsz],
                                 start=True, stop=True)

            r_sb = mids.tile([H, Q], bf16, tag="r_sb", bufs=3, name="r_sb")[:, :qsz]
            z_sb = mids.tile([H, Q], bf16, tag="z_sb", bufs=3, name="z_sb")[:, :qsz]
            nc.scalar.activation(out=r_sb, in_=r_ps, func=sig,
                                 bias=b_sb[:, 0:1], scale=1.0)
            nc.scalar.activation(out=z_sb, in_=z_ps, func=sig,
                                 bias=b_sb[:, 1:2], scale=1.0)

            # t = (h_n + b_hhn) * r written IN PLACE into the C bank; the
            # delayed i_n matmul then accumulates on top of it.
            nc.vector.scalar_tensor_tensor(
                out=c_ps, in0=c_ps, scalar=b_sb[:, 2:3], in1=r_sb,
                op0=add_op, op1=mult_op)

            state[qi] = (c_ps, z_sb, halves, x_sb, h_sb, xo)

        def emit_back(qi):
            """Delayed i_n accumulation + tanh + blend + out DMA for qi."""
            q0, qsz = qs[qi]
            c_ps, z_sb, halves, x_sb, h_sb, xo = state.pop(qi)

            for h0, hsz in halves:
                nc.tensor.matmul(c_ps[:, h0 - xo : h0 - xo + hsz],
                                 w_ih_sb[:, W_N], x_sb[:, h0 : h0 + hsz],
                                 start=False, stop=True, skip_group_check=True)

            n_sb = mids.tile([H, Q], bf16, tag="n_sb", bufs=3, name="n_sb")[:, :qsz]
            nc.scalar.activation(out=n_sb, in_=c_ps, func=tanh,
                                 bias=b_sb[:, 3:4], scale=1.0)

            oc, oin = oq[qi]
            o_sb = wide.tile([H, Q], bf16, tag="o", bufs=3,
                             name="o_sb")[:, :qsz]

            # o = n + z*(h-n) as three all-bf16 tensor_tensor ops (DVE 2x
            # rate); the subtract alternates onto GpSimd to offload DVE
            d_sb = mids.tile([H, Q], bf16, tag="d", bufs=3, name="d_sb")[:, :qsz]
            m_sb = mids.tile([H, Q], bf16, tag="m", bufs=3, name="m_sb")[:, :qsz]
            d_eng = nc.gpsimd if (qi % 2 == 0) else nc.vector
            d_eng.tensor_sub(out=d_sb, in0=h_sb[:, xo : xo + qsz], in1=n_sb)
            nc.vector.tensor_mul(out=m_sb, in0=z_sb, in1=d_sb)
            nc.vector.tensor_add(out=o_sb[:, oin : oin + qsz], in0=m_sb,
                                 in1=n_sb)

            nc.sync.dma_start(out=outT[:, q0 : q0 + qsz], in_=o_sb)

        for qi in range(nq):
            if qi > 0:
                emit_back(qi - 1)
            emit_front(qi)
        emit_back(nq - 1)

    _patch_json(nc)
    return nc


def _get_nc(bpc: int) -> bass.Bass:
    if bpc not in _NC_CACHE:
        _NC_CACHE[bpc] = _build_nc(bpc)
    return _NC_CACHE[bpc]


def kernel(node_ids, messages, memory, W_ih, W_hh, b_ih, b_hh):
    global LAST_RESULT
    node_ids = np.asarray(node_ids)
    messages = np.asarray(messages, dtype=np.float32)
    memory = np.asarray(memory, dtype=np.float32)
    W_ih = np.asarray(W_ih, dtype=np.float32)
    W_hh = np.asarray(W_hh, dtype=np.float32)
    b_ih = np.asarray(b_ih, dtype=np.float32)
    b_hh = np.asarray(b_hh, dtype=np.float32)

    B = node_ids.shape[0]
    per = -(-B // N_CORES)                       # rows per core
    bpc = max(per, 512)
    nc = _get_nc(bpc)

    current = memory[node_ids]                   # [B, H] host gather

    w_ihT = np.ascontiguousarray(W_ih.T).astype(BF16)
    w_hhT = np.ascontiguousarray(W_hh.T).astype(BF16)
    bias = np.empty((H, 4), dtype=np.float32)
    bias[:, 0] = b_ih[0:H] + b_hh[0:H]
    bias[:, 1] = b_ih[H : 2 * H] + b_hh[H : 2 * H]
    bias[:, 2] = b_hh[2 * H : 3 * H]
    bias[:, 3] = b_ih[2 * H : 3 * H]

    in_maps = []
    for c in range(N_CORES):
        lo = c * per
        hi = min(lo + per, B)
        if hi - lo == bpc:
            xT = np.ascontiguousarray(messages[lo:hi].T).astype(BF16)
            hT = np.ascontiguousarray(current[lo:hi].T).astype(BF16)
        else:
            xT = np.zeros((H, bpc), dtype=BF16)
            hT = np.zeros((H, bpc), dtype=BF16)
            if hi > lo:
                xT[:, : hi - lo] = messages[lo:hi].T
                hT[:, : hi - lo] = current[lo:hi].T
        in_maps.append({
            "xT": xT, "hT": hT,
            "w_ihT": w_ihT, "w_hhT": w_hhT, "biases": bias,
        })

    res = run_bass_kernel_spmd(nc, in_maps, list(range(N_CORES)))
    LAST_RESULT = res

    updated = np.empty((B, H), dtype=np.float32)
    for c in range(N_CORES):
        lo = c * per
        hi = min(lo + per, B)
        if hi > lo:
            updated[lo:hi] = res.results[c]["outT"][:, : hi - lo].T.astype(np.float32)

    new_memory = memory.copy()
    new_memory[node_ids] = updated
    return new_memory
